# revision 8
# baseline (speedup 1.0000x reference)
"""AGF sparse attention (top-k=64 mask + softmax + 3-term polynomial filter)
on 8 TRN2 NeuronCores.

Sharding: core c -> batch b = c//2, head-group hg = c%2 (4 of 8 heads).
Each core runs the full per-(b, head-group) pipeline on-device.

v2 layout (vs v1): scores stay in PSUM; Scalar drains them straight to
E = exp(SCALE*S - 8) f16 (constant bias keeps E in f16 range, softmax
normalization cancels it). Top-64 threshold runs on E (exp is monotone):
32x max8 over 64-wide chunks -> 256 candidates -> 8x(max8+match_replace)
rounds; tau = 64th largest E directly (no exp/reduce/negate ops). DVE also
does the fused mask+rowsum STT and the reciprocal; everything else moved
off DVE: res accumulation is a GpSimd (Pool) tensor_tensor ADD of the
alpha-folded z drains (alpha ratios folded into the per-step 1/rowsum
drain scales), res_row/out-projection run in f16.

Engine budget: DVE = top-k + mask only; Act = exp drains + z drains +
projections' psum drains; Pool = res accumulation (+memset); PE = matmuls;
DMA = A^T transposes via DRAM roundtrip + input/output traffic.
"""

import sys

sys.path.insert(0, "/opt/trn_rl_repo")

from contextlib import ExitStack  # noqa: E402

import numpy as np  # noqa: E402

import concourse.bass as bass  # noqa: E402
import concourse.tile as tile  # noqa: E402
from concourse import bacc, mybir  # noqa: E402
from concourse.bass_utils import run_bass_kernel_spmd  # noqa: E402

FP = mybir.dt.float32
FPR = mybir.dt.float32r
F16 = mybir.dt.float16
AF = mybir.ActivationFunctionType
ALU = mybir.AluOpType

N, DIM = 2048, 512
H, HL, DH = 8, 4, 64  # total heads, local heads per core, head dim
NT = N // 128  # 16 token tiles
CC = DIM // 128  # 4 contraction chunks
ORDER = 3
NEG = -60000.0  # must fit fp16
SCALE = DH**-0.5  # 0.125
EBIAS = -8.0  # constant exp bias; cancels in softmax, keeps E <= e^1 in f16
NCH = 32  # top-k candidate chunks per row (chunk width = N // NCH = 64)


def _build():
    nc = bacc.Bacc(
        "TRN2", target_bir_lowering=False, debug=False, num_devices=8
    )
    xt_d = nc.dram_tensor("xt", [DIM, N], FP, kind="ExternalInput")
    wqk_d = nc.dram_tensor("wqk", [DIM, 512], FP, kind="ExternalInput")
    wv_d = nc.dram_tensor("wv", [DIM, 256], FP, kind="ExternalInput")
    bqk_d = nc.dram_tensor("bqk", [512], FP, kind="ExternalInput")
    bv_d = nc.dram_tensor("bv", [1, 256], FP, kind="ExternalInput")
    wout_d = nc.dram_tensor("wout", [256, DIM], FP, kind="ExternalInput")
    bout_d = nc.dram_tensor("bout", [1, DIM], FP, kind="ExternalInput")
    ar_d = nc.dram_tensor("araw", [1, ORDER * HL], FP, kind="ExternalInput")
    out_d = nc.dram_tensor("out", [N, DIM], FP, kind="ExternalOutput")

    ident_bf_d = nc.inline_tensor(
        np.eye(128, dtype=np.float16), name="identbf"
    )
    ones_d = nc.inline_tensor(np.ones((1, 128), np.float32), name="ones1")
    half_d = nc.inline_tensor(
        np.full((1, 128), 0.5, np.float32), name="half1"
    )

    with tile.TileContext(nc) as tc, ExitStack() as ctx:
        consts = ctx.enter_context(tc.tile_pool(name="consts", bufs=1))
        pw = ctx.enter_context(tc.tile_pool(name="weights", bufs=1))
        pqk = ctx.enter_context(tc.tile_pool(name="qkT", bufs=1))
        pv = ctx.enter_context(tc.tile_pool(name="vsb", bufs=1))
        pres = ctx.enter_context(tc.tile_pool(name="res", bufs=1))
        psum = ctx.enter_context(tc.tile_pool(name="psum", bufs=2, space="PSUM"))
        pS = ctx.enter_context(tc.tile_pool(name="pS", bufs=2))
        pA = ctx.enter_context(tc.tile_pool(name="pA", bufs=8))
        pzz = ctx.enter_context(tc.tile_pool(name="pzz", bufs=2))
        pAD = ctx.enter_context(
            tc.tile_pool(name="adram", bufs=2, space="DRAM")
        )

        ident_bf = consts.tile([128, 128], F16)
        nc.sync.dma_start(ident_bf[:], ident_bf_d.ap())
        ones_r = consts.tile([1, 128], FPR)
        nc.sync.dma_start(ones_r[:], ones_d.ap().bitcast(FPR))
        half_f = consts.tile([1, 128], FP)
        nc.sync.dma_start(half_f[:], half_d.ap())
        bvrow = consts.tile([1, 256], FPR)
        nc.sync.dma_start(bvrow[:], bv_d.ap().bitcast(FPR))
        boutrow = consts.tile([1, 512], FP)
        nc.sync.dma_start(boutrow[:], bout_d.ap())
        bqk_sb = consts.tile([128, 4], FP)
        nc.sync.dma_start(
            bqk_sb[:], bqk_d.ap().rearrange("(f p) -> p f", p=128)
        )
        araw_t = consts.tile([1, ORDER * HL], FP)
        nc.sync.dma_start(araw_t[:], ar_d.ap())
        alpha_g = consts.tile([1, ORDER * HL], FP)
        nc.scalar.activation(alpha_g[:], araw_t[:], AF.Gelu)
        # drain-scale ratios: k[0,h]=a0; k[r,h]=a_r/a_{r-1} (r-major layout)
        arecip = consts.tile([1, 2 * HL], FP)
        nc.vector.reciprocal(arecip[:], alpha_g[:, 0 : 2 * HL])
        kg = consts.tile([1, ORDER * HL], FP)
        nc.scalar.copy(kg[:, 0:HL], alpha_g[:, 0:HL])
        nc.vector.tensor_tensor(
            kg[:, HL : 3 * HL],
            alpha_g[:, HL : 3 * HL],
            arecip[:],
            op=ALU.mult,
        )
        kbc = consts.tile([128, ORDER * HL], FP)
        nc.gpsimd.partition_broadcast(kbc[:], kg[:])
        ebias = consts.tile([128, 1], FP)
        nc.vector.memset(ebias[:], EBIAS)

        wqk_sb = []
        wv_sb = []
        for c in range(CC):
            t = pw.tile([128, 512], FPR, tag=f"wqk{c}", name=f"wqk{c}")
            nc.sync.dma_start(t[:], wqk_d.ap().bitcast(FPR)[c * 128 : (c + 1) * 128, :])
            wqk_sb.append(t)
            t = pw.tile([128, 256], FPR, tag=f"wv{c}", name=f"wv{c}")
            nc.sync.dma_start(t[:], wv_d.ap().bitcast(FPR)[c * 128 : (c + 1) * 128, :])
            wv_sb.append(t)
        # out-projection weights converted to f16 on device
        wout_sb = []
        for f in range(2):
            t = pw.tile([128, 512], F16, tag=f"wo{f}", name=f"wo{f}")
            wout_sb.append(t)
        half16 = consts.tile([1, 128], F16)
        nc.scalar.copy(half16[:], half_f[:])
        bout16 = consts.tile([1, 512], F16)
        nc.scalar.copy(bout16[:], boutrow[:])

        # qkT tiles: ft 0..1 = q^T (heads 0-1, 2-3), ft 2..3 = k^T
        qkT = [
            pqk.tile([128, N], F16, tag=f"qkT{i}", name=f"qkT{i}")
            for i in range(4)
        ]
        v_sb = pv.tile([128, NT, 256], F16)  # v rows, packed [t_lo, t_hi, f]
        res_row = pres.tile([128, NT, 256], F16)  # sum_r alpha_r z_r (rows)
        nc.gpsimd.memset(res_row[:], 0.0)

        # ---- phase 1: load x^T; phase 2: projections (all fp32r)
        with tc.tile_pool(name="xload", bufs=1) as px:
            xT = [
                px.tile([128, N], FPR, tag=f"xT{c}", name=f"xT{c}")
                for c in range(CC)
            ]
            for c in range(CC):
                nc.sync.dma_start(
                    xT[c][:],
                    xt_d.ap().bitcast(FPR)[c * 128 : (c + 1) * 128, :],
                )
            for f in range(2):
                tf32 = px.tile([128, 512], FP, tag=f"wo32{f}", name=f"wo32{f}")
                nc.sync.dma_start(
                    tf32[:], wout_d.ap()[f * 128 : (f + 1) * 128, :]
                )
                nc.scalar.copy(wout_sb[f][:], tf32[:])

            for ft in (0, 2, 1, 3):
                for tq in range(4):
                    ps = psum.tile([128, 512], FP, tag="gen", name="psg")
                    for c in range(CC):
                        nc.tensor.matmul(
                            ps[:],
                            wqk_sb[c][:, ft * 128 : (ft + 1) * 128],
                            xT[c][:, tq * 512 : (tq + 1) * 512],
                            start=(c == 0),
                            stop=(c == CC - 1),
                        )
                    nc.scalar.activation(
                        qkT[ft][:, tq * 512 : (tq + 1) * 512],
                        ps[:],
                        AF.Identity,
                        bias=bqk_sb[:, ft : ft + 1],
                    )
            for it in range(NT):
                psv = psum.tile([128, 512], FP, tag="gen", name="psg")
                ps = psv[:, 0:256]
                nc.tensor.matmul(
                    ps,
                    ones_r[:],
                    bvrow[:],
                    start=True,
                    stop=False,
                )
                for c in range(CC):
                    nc.tensor.matmul(
                        ps,
                        xT[c][:, it * 128 : (it + 1) * 128],
                        wv_sb[c][:],
                        start=False,
                        stop=(c == CC - 1),
                    )
                nc.scalar.copy(v_sb[:, it, :], ps)

        # ---- head loop
        pout = ctx.enter_context(tc.tile_pool(name="pout", bufs=1))
        resT = [
            pout.tile([128, N], F16, tag=f"rT{f}", name=f"rT{f}")
            for f in range(2)
        ]
        out_acc = pout.tile([128, NT, 512], F16)  # partial (bias + heads 0-1)

        def out_half(f):
            # transpose res halves for heads [2f, 2f+1] and run their part
            # of the output projection; f=0 seeds with 0.5*bout, f=1 reloads
            # the partial via a PE identity matmul and writes the result.
            for g in range(4):
                ps = psum.tile([128, 512], F16, tag="gen", name="psg")
                for u in range(4):
                    jt = g * 4 + u
                    nc.tensor.transpose(
                        ps[:, u * 128 : (u + 1) * 128],
                        res_row[:, jt, f * 128 : (f + 1) * 128],
                        ident_bf[:],
                    )
                nc.scalar.copy(resT[f][:, g * 512 : (g + 1) * 512], ps[:])
            for tt in range(NT):
                ps = psum.tile([128, 512], FP, tag="gen", name="psg")
                if f == 0:
                    nc.tensor.matmul(
                        ps[:], half16[:], bout16[:], start=True, stop=False
                    )
                else:
                    nc.tensor.matmul(
                        ps[:], ident_bf[:], out_acc[:, tt, :],
                        start=True, stop=False,
                    )
                nc.tensor.matmul(
                    ps[:],
                    resT[f][:, tt * 128 : (tt + 1) * 128],
                    wout_sb[f][:],
                    start=False,
                    stop=True,
                )
                if f == 0:
                    nc.scalar.copy(out_acc[:, tt, :], ps[:])
                else:
                    o_sb = pS.tile([128, 512], FP, tag="osb", bufs=2)
                    nc.scalar.copy(o_sb[:], ps[:])
                    nc.sync.dma_start(
                        out_d.ap()[tt * 128 : (tt + 1) * 128, :], o_sb[:]
                    )

        with tc.tile_pool(name="pAT", bufs=1) as pAT:
            for hl in range(HL):
                qt_tile = qkT[hl // 2]
                kt_tile = qkT[2 + hl // 2]
                po = (hl % 2) * 64
                AT = pAT.tile([128, NT, N], F16, tag="AT")
                A_dram = pAD.tile([N, N], F16, tag="ad", name="adram")
                rd_all = pzz.tile([128, NT], FP, tag="rd")
                import contextlib
                for it in range(NT):
                    hoist = (
                        tc.high_priority() if hl == 0
                        else contextlib.nullcontext()
                    )
                    E_sb = pS.tile([128, N], F16, tag="E", bufs=3)
                    with hoist:
                        for jc in range(4):
                            ps = psum.tile(
                                [128, 512], FP, tag="S", bufs=3, name="psS"
                            )
                            nc.tensor.matmul(
                                ps[:],
                                qt_tile[
                                    po : po + 64, it * 128 : (it + 1) * 128
                                ],
                                kt_tile[
                                    po : po + 64, jc * 512 : (jc + 1) * 512
                                ],
                                start=True,
                                stop=True,
                            )
                            # E = exp(SCALE*S - 8) straight from PSUM
                            nc.scalar.activation(
                                E_sb[:, jc * 512 : (jc + 1) * 512],
                                ps[:],
                                AF.Exp,
                                bias=ebias[:, 0:1],
                                scale=SCALE,
                            )
                    # top-64 threshold on E (exp monotone): 32 chunks of 64
                    # -> top-8 candidates each, then 8x(max8 + match_replace)
                    C = pS.tile([128, NCH * 8], F16, tag="C", bufs=5)
                    chw = N // NCH
                    for ch in range(NCH):
                        nc.vector.max(
                            C[:, ch * 8 : (ch + 1) * 8],
                            E_sb[:, ch * chw : (ch + 1) * chw],
                        )
                    m8 = pS.tile([128, 8], F16, tag="m8", bufs=5)
                    for itr in range(8):
                        nc.vector.max(m8[:], C[:])
                        if itr < 7:
                            nc.vector.match_replace(C[:], m8[:], C[:], NEG)
                    A_bf = pA.tile([128, N], F16, tag="A")
                    dsum = pS.tile([128, 1], FP, tag="d", bufs=5)
                    nc.vector.scalar_tensor_tensor(
                        A_bf[:],
                        E_sb[:],
                        m8[:, 7:8],
                        E_sb[:],
                        op0=ALU.is_ge,
                        op1=ALU.mult,
                        accum_out=dsum[:],
                    )
                    nc.vector.reciprocal(rd_all[:, it : it + 1], dsum[:])
                    nc.sync.dma_start(
                        A_dram[it * 128 : (it + 1) * 128, :], A_bf[:]
                    )
                    nc.sync.dma_start_transpose(
                        AT[:, :, it * 128 : (it + 1) * 128],
                        A_dram[it * 128 : (it + 1) * 128, :],
                    )
                # recurrence: z_r = rda * (A_u @ z_{r-1}), res += z drains
                # (alpha ratios folded into the drain scales)
                zprev = None
                for r in range(ORDER):
                    rda = pzz.tile([128, NT], FP, tag=f"rda{r % 2}")
                    nc.vector.tensor_scalar_mul(
                        rda[:], rd_all[:],
                        kbc[:, r * HL + hl : r * HL + hl + 1],
                    )
                    ztmp = pzz.tile([64, N], F16, tag="ztmp")
                    for nck in range(4):
                        psz = psum.tile(
                            [64, 512], FP, tag="z", bufs=2, name="psz"
                        )
                        for jt in range(NT):
                            lhsT = (
                                v_sb[:, jt, hl * 64 : (hl + 1) * 64]
                                if r == 0
                                else zprev[:, jt, :]
                            )
                            nc.tensor.matmul(
                                psz[:],
                                lhsT,
                                AT[:, jt, nck * 512 : (nck + 1) * 512],
                                start=(jt == 0),
                                stop=(jt == NT - 1),
                            )
                        nc.scalar.copy(
                            ztmp[:, nck * 512 : (nck + 1) * 512], psz[:]
                        )
                    # znew_r = (k_r * rd) * z^T transposed back; the k_r
                    # ratio makes znew_r = alpha_{r} * attn^{r+1} v when
                    # chained, so res accumulation is a plain add.
                    znew = pzz.tile(
                        [128, NT, 64], F16, tag=f"z{r % 2}", name=f"znew{r}"
                    )
                    pzt = psum.tile(
                        [128, 1024], F16, tag="zt", bufs=1, name="pzt"
                    )
                    for jt in range(NT):
                        nc.tensor.transpose(
                            pzt[:, jt * 64 : (jt + 1) * 64],
                            ztmp[:, jt * 128 : (jt + 1) * 128],
                            ident_bf[0:64, 0:64],
                        )
                    for jt in range(NT):
                        nc.scalar.activation(
                            znew[:, jt, :],
                            pzt[:, jt * 64 : (jt + 1) * 64],
                            AF.Copy,
                            scale=rda[:, jt : jt + 1],
                        )
                    rslice = res_row[:, :, hl * 64 : (hl + 1) * 64]
                    nc.gpsimd.tensor_tensor(
                        rslice, rslice, znew[:], op=ALU.add
                    )
                    # next step's lhsT must be rd*z (no alpha); the alpha
                    # factor is divided back out by the k_{r+1} ratio.
                    zprev = znew
                if hl == 1:
                    out_half(0)
            out_half(1)

    nc.compile()
    return nc


_CACHE: dict = {}


def _in_maps(x, Wqkv, bqkv, Wout, bout, alphas_raw):
    maps = []
    for c in range(8):
        b, hg = c // 2, c % 2
        s, e = hg * 256, (hg + 1) * 256
        wqk = np.concatenate(
            [Wqkv[:, s:e], Wqkv[:, 512 + s : 512 + e]], axis=1
        )
        maps.append(
            {
                "xt": np.ascontiguousarray(x[b].T, np.float32),
                "wqk": np.ascontiguousarray(wqk, np.float32),
                "wv": np.ascontiguousarray(
                    Wqkv[:, 1024 + s : 1024 + e], np.float32
                ),
                "bqk": np.ascontiguousarray(
                    np.concatenate([bqkv[s:e], bqkv[512 + s : 512 + e]]),
                    np.float32,
                ),
                "bv": np.ascontiguousarray(
                    bqkv[None, 1024 + s : 1024 + e], np.float32
                ),
                "wout": np.ascontiguousarray(Wout[s:e, :], np.float32),
                "bout": np.ascontiguousarray(bout[None, :], np.float32),
                "araw": np.ascontiguousarray(
                    alphas_raw[:, hg * HL : (hg + 1) * HL].reshape(1, -1),
                    np.float32,
                ),
            }
        )
    return maps


def kernel(x, Wqkv, bqkv, Wout, bout, alphas_raw, _trace=False):
    x = np.asarray(x, np.float32)
    if "nc" not in _CACHE:
        _CACHE["nc"] = _build()
    nc = _CACHE["nc"]
    maps = _in_maps(
        np.asarray(x), np.asarray(Wqkv), np.asarray(bqkv),
        np.asarray(Wout), np.asarray(bout), np.asarray(alphas_raw),
    )
    kw = {}
    if _trace:
        kw = {"trace": True}
    res = run_bass_kernel_spmd(nc, maps, core_ids=list(range(8)), **kw)
    _CACHE["last_results"] = res
    out = np.empty((4, N, DIM), np.float32)
    for b in range(4):
        out[b] = res.results[2 * b]["out"] + res.results[2 * b + 1]["out"]
    return out


# revision 20
# speedup vs baseline: 1.0140x; 1.0140x over previous
"""AGF sparse attention (top-k=64 mask + softmax + 3-term polynomial filter)
on 8 TRN2 NeuronCores.

Sharding: core c -> batch b = c//2, head-group hg = c%2 (4 of 8 heads).
Each core runs the full per-(b, head-group) pipeline on-device.

v2 layout (vs v1): scores stay in PSUM; Scalar drains them straight to
E = exp(SCALE*S - 8) f16 (constant bias keeps E in f16 range, softmax
normalization cancels it). Top-64 threshold runs on E (exp is monotone):
32x max8 over 64-wide chunks -> 256 candidates -> 8x(max8+match_replace)
rounds; tau = 64th largest E directly (no exp/reduce/negate ops). DVE also
does the fused mask+rowsum STT and the reciprocal; everything else moved
off DVE: res accumulation is a GpSimd (Pool) tensor_tensor ADD of the
alpha-folded z drains (alpha ratios folded into the per-step 1/rowsum
drain scales), res_row/out-projection run in f16.

Engine budget: DVE = top-k + mask only; Act = exp drains + z drains +
projections' psum drains; Pool = res accumulation (+memset); PE = matmuls;
DMA = A^T transposes via DRAM roundtrip + input/output traffic.
"""

import sys

sys.path.insert(0, "/opt/trn_rl_repo")

from contextlib import ExitStack  # noqa: E402

import numpy as np  # noqa: E402

import concourse.bass as bass  # noqa: E402
import concourse.tile as tile  # noqa: E402
from concourse import bacc, mybir  # noqa: E402
from concourse.bass_utils import run_bass_kernel_spmd  # noqa: E402

FP = mybir.dt.float32
FPR = mybir.dt.float32r
F16 = mybir.dt.float16
AF = mybir.ActivationFunctionType
ALU = mybir.AluOpType

N, DIM = 2048, 512
H, HL, DH = 8, 4, 64  # total heads, local heads per core, head dim
NT = N // 128  # 16 token tiles
CC = DIM // 128  # 4 contraction chunks
ORDER = 3
NEG = -60000.0  # must fit fp16
SCALE = DH**-0.5  # 0.125
EBIAS = -8.0  # constant exp bias; cancels in softmax, keeps E <= e^1 in f16
NCH = 32  # top-k candidate chunks per row (chunk width = N // NCH = 64)


def _build():
    nc = bacc.Bacc(
        "TRN2", target_bir_lowering=False, debug=False, num_devices=8
    )
    xt_d = nc.dram_tensor("xt", [DIM, N], FP, kind="ExternalInput")
    wqk_d = nc.dram_tensor("wqk", [DIM, 512], FP, kind="ExternalInput")
    wv_d = nc.dram_tensor("wv", [DIM, 256], FP, kind="ExternalInput")
    bqk_d = nc.dram_tensor("bqk", [512], FP, kind="ExternalInput")
    bv_d = nc.dram_tensor("bv", [1, 256], FP, kind="ExternalInput")
    wout_d = nc.dram_tensor("wout", [256, DIM], FP, kind="ExternalInput")
    bout_d = nc.dram_tensor("bout", [1, DIM], FP, kind="ExternalInput")
    ar_d = nc.dram_tensor("araw", [1, ORDER * HL], FP, kind="ExternalInput")
    out_d = nc.dram_tensor("out", [N, DIM], FP, kind="ExternalOutput")

    ident_bf_d = nc.inline_tensor(
        np.eye(128, dtype=np.float16), name="identbf"
    )
    ones_d = nc.inline_tensor(np.ones((1, 128), np.float32), name="ones1")
    half_d = nc.inline_tensor(
        np.full((1, 128), 0.5, np.float32), name="half1"
    )

    with tile.TileContext(nc) as tc, ExitStack() as ctx:
        consts = ctx.enter_context(tc.tile_pool(name="consts", bufs=1))
        pw = ctx.enter_context(tc.tile_pool(name="weights", bufs=1))
        pqk = ctx.enter_context(tc.tile_pool(name="qkT", bufs=1))
        pv = ctx.enter_context(tc.tile_pool(name="vsb", bufs=1))
        pres = ctx.enter_context(tc.tile_pool(name="res", bufs=1))
        psum = ctx.enter_context(tc.tile_pool(name="psum", bufs=2, space="PSUM"))
        pS = ctx.enter_context(tc.tile_pool(name="pS", bufs=2))
        pA = ctx.enter_context(tc.tile_pool(name="pA", bufs=8))
        pzz = ctx.enter_context(tc.tile_pool(name="pzz", bufs=2))
        pAD = ctx.enter_context(
            tc.tile_pool(name="adram", bufs=2, space="DRAM")
        )

        ident_bf = consts.tile([128, 128], F16)
        nc.sync.dma_start(ident_bf[:], ident_bf_d.ap())
        ones_r = consts.tile([1, 128], FPR)
        nc.sync.dma_start(ones_r[:], ones_d.ap().bitcast(FPR))
        half_f = consts.tile([1, 128], FP)
        nc.sync.dma_start(half_f[:], half_d.ap())
        bvrow = consts.tile([1, 256], FPR)
        nc.sync.dma_start(bvrow[:], bv_d.ap().bitcast(FPR))
        boutrow = consts.tile([1, 512], FP)
        nc.sync.dma_start(boutrow[:], bout_d.ap())
        bqk_sb = consts.tile([128, 4], FP)
        nc.sync.dma_start(
            bqk_sb[:], bqk_d.ap().rearrange("(f p) -> p f", p=128)
        )
        araw_t = consts.tile([1, ORDER * HL], FP)
        nc.sync.dma_start(araw_t[:], ar_d.ap())
        alpha_g = consts.tile([1, ORDER * HL], FP)
        nc.scalar.activation(alpha_g[:], araw_t[:], AF.Gelu)
        # drain-scale ratios: k[0,h]=a0; k[r,h]=a_r/a_{r-1} (r-major layout)
        arecip = consts.tile([1, 2 * HL], FP)
        nc.vector.reciprocal(arecip[:], alpha_g[:, 0 : 2 * HL])
        kg = consts.tile([1, ORDER * HL], FP)
        nc.scalar.copy(kg[:, 0:HL], alpha_g[:, 0:HL])
        nc.vector.tensor_tensor(
            kg[:, HL : 3 * HL],
            alpha_g[:, HL : 3 * HL],
            arecip[:],
            op=ALU.mult,
        )
        kbc = consts.tile([128, ORDER * HL], FP)
        nc.gpsimd.partition_broadcast(kbc[:], kg[:])
        ebias = consts.tile([128, 1], FP)
        nc.vector.memset(ebias[:], EBIAS)

        wqk_sb = []
        wv_sb = []
        for c in range(CC):
            t = pw.tile([128, 512], FPR, tag=f"wqk{c}", name=f"wqk{c}")
            nc.sync.dma_start(t[:], wqk_d.ap().bitcast(FPR)[c * 128 : (c + 1) * 128, :])
            wqk_sb.append(t)
            t = pw.tile([128, 256], FPR, tag=f"wv{c}", name=f"wv{c}")
            nc.sync.dma_start(t[:], wv_d.ap().bitcast(FPR)[c * 128 : (c + 1) * 128, :])
            wv_sb.append(t)
        # out-projection weights converted to f16 on device
        wout_sb = []
        for f in range(2):
            t = pw.tile([128, 512], F16, tag=f"wo{f}", name=f"wo{f}")
            wout_sb.append(t)
        half16 = consts.tile([1, 128], F16)
        nc.scalar.copy(half16[:], half_f[:])
        bout16 = consts.tile([1, 512], F16)
        nc.scalar.copy(bout16[:], boutrow[:])

        # qkT tiles: ft 0..1 = q^T (heads 0-1, 2-3), ft 2..3 = k^T
        qkT = [
            pqk.tile([128, N], F16, tag=f"qkT{i}", name=f"qkT{i}")
            for i in range(4)
        ]
        v_sb = pv.tile([128, NT, 256], F16)  # v rows, packed [t_lo, t_hi, f]
        res_row = pres.tile([128, NT, 256], F16)  # sum_r alpha_r z_r (rows)
        nc.gpsimd.memset(res_row[:], 0.0)

        # ---- phase 1: load x^T; phase 2: projections (all fp32r)
        with tc.tile_pool(name="xload", bufs=1) as px:
            xT = [
                px.tile([128, N], FPR, tag=f"xT{c}", name=f"xT{c}")
                for c in range(CC)
            ]
            for c in range(CC):
                nc.sync.dma_start(
                    xT[c][:],
                    xt_d.ap().bitcast(FPR)[c * 128 : (c + 1) * 128, :],
                )
            for f in range(2):
                tf32 = px.tile([128, 512], FP, tag=f"wo32{f}", name=f"wo32{f}")
                nc.sync.dma_start(
                    tf32[:], wout_d.ap()[f * 128 : (f + 1) * 128, :]
                )
                nc.scalar.copy(wout_sb[f][:], tf32[:])

            for ft in (2, 0, 3, 1):
                for tq in range(4):
                    ps = psum.tile([128, 512], FP, tag="gen", name="psg")
                    for c in range(CC):
                        nc.tensor.matmul(
                            ps[:],
                            wqk_sb[c][:, ft * 128 : (ft + 1) * 128],
                            xT[c][:, tq * 512 : (tq + 1) * 512],
                            start=(c == 0),
                            stop=(c == CC - 1),
                        )
                    nc.scalar.activation(
                        qkT[ft][:, tq * 512 : (tq + 1) * 512],
                        ps[:],
                        AF.Identity,
                        bias=bqk_sb[:, ft : ft + 1],
                    )
            for it in range(NT):
                psv = psum.tile([128, 512], FP, tag="gen", name="psg")
                ps = psv[:, 0:256]
                nc.tensor.matmul(
                    ps,
                    ones_r[:],
                    bvrow[:],
                    start=True,
                    stop=False,
                )
                for c in range(CC):
                    nc.tensor.matmul(
                        ps,
                        xT[c][:, it * 128 : (it + 1) * 128],
                        wv_sb[c][:],
                        start=False,
                        stop=(c == CC - 1),
                    )
                nc.scalar.copy(v_sb[:, it, :], ps)

        # ---- head loop
        pout = ctx.enter_context(tc.tile_pool(name="pout", bufs=1))
        resT = [
            pout.tile([128, N], F16, tag=f"rT{f}", name=f"rT{f}")
            for f in range(2)
        ]
        out_acc = pout.tile([128, NT, 512], F16)  # partial (bias + heads 0-1)

        def out_transpose(f, gs, ge):
            # transpose res column-halves for head-pair f, groups [gs, ge)
            for g in range(gs, ge):
                ps = psum.tile([128, 512], F16, tag="gen", name="psg")
                for u in range(4):
                    jt = g * 4 + u
                    nc.tensor.transpose(
                        ps[:, u * 128 : (u + 1) * 128],
                        res_row[:, jt, f * 128 : (f + 1) * 128],
                        ident_bf[:],
                    )
                nc.scalar.copy(resT[f][:, g * 512 : (g + 1) * 512], ps[:])

        def out_matmul(f, ts, te):
            # f=0 seeds with 0.5*bout into out_acc; f=1 reloads the partial
            # via a PE identity matmul and writes the final result.
            for tt in range(ts, te):
                ps = psum.tile([128, 512], FP, tag="gen", name="psg")
                if f == 0:
                    nc.tensor.matmul(
                        ps[:], half16[:], bout16[:], start=True, stop=False
                    )
                else:
                    nc.tensor.matmul(
                        ps[:], ident_bf[:], out_acc[:, tt, :],
                        start=True, stop=False,
                    )
                nc.tensor.matmul(
                    ps[:],
                    resT[f][:, tt * 128 : (tt + 1) * 128],
                    wout_sb[f][:],
                    start=False,
                    stop=True,
                )
                if f == 0:
                    nc.scalar.copy(out_acc[:, tt, :], ps[:])
                else:
                    o_sb = pS.tile([128, 512], FP, tag="osb", bufs=2)
                    nc.scalar.copy(o_sb[:], ps[:])
                    nc.sync.dma_start(
                        out_d.ap()[tt * 128 : (tt + 1) * 128, :], o_sb[:]
                    )

        with tc.tile_pool(name="pAT", bufs=1) as pAT:
            for hl in range(HL):
                qt_tile = qkT[hl // 2]
                kt_tile = qkT[2 + hl // 2]
                po = (hl % 2) * 64
                AT = pAT.tile([128, NT, N], F16, tag="AT")
                A_dram = pAD.tile([N, N], F16, tag="ad", name="adram")
                rd_all = pzz.tile([128, NT], FP, tag="rd")
                import contextlib
                for it in range(NT):
                    hoist = (
                        tc.high_priority() if hl == 0
                        else contextlib.nullcontext()
                    )
                    E_sb = pS.tile([128, N], F16, tag="E", bufs=3)
                    with hoist:
                        for jc in range(4):
                            ps = psum.tile(
                                [128, 512], FP, tag="S", bufs=3, name="psS"
                            )
                            nc.tensor.matmul(
                                ps[:],
                                qt_tile[
                                    po : po + 64, it * 128 : (it + 1) * 128
                                ],
                                kt_tile[
                                    po : po + 64, jc * 512 : (jc + 1) * 512
                                ],
                                start=True,
                                stop=True,
                            )
                            # E = exp(SCALE*S - 8) straight from PSUM
                            nc.scalar.activation(
                                E_sb[:, jc * 512 : (jc + 1) * 512],
                                ps[:],
                                AF.Exp,
                                bias=ebias[:, 0:1],
                                scale=SCALE,
                            )
                    # top-64 threshold on E (exp monotone): 32 chunks of 64
                    # -> top-8 candidates each, then 8x(max8 + match_replace)
                    C = pS.tile([128, NCH * 8], F16, tag="C", bufs=5)
                    chw = N // NCH
                    for ch in range(NCH):
                        nc.vector.max(
                            C[:, ch * 8 : (ch + 1) * 8],
                            E_sb[:, ch * chw : (ch + 1) * chw],
                        )
                    m8 = pS.tile([128, 8], F16, tag="m8", bufs=5)
                    for itr in range(8):
                        nc.vector.max(m8[:], C[:])
                        if itr < 7:
                            nc.vector.match_replace(C[:], m8[:], C[:], NEG)
                    A_bf = pA.tile([128, N], F16, tag="A")
                    dsum = pS.tile([128, 1], FP, tag="d", bufs=5)
                    nc.vector.scalar_tensor_tensor(
                        A_bf[:],
                        E_sb[:],
                        m8[:, 7:8],
                        E_sb[:],
                        op0=ALU.is_ge,
                        op1=ALU.mult,
                        accum_out=dsum[:],
                    )
                    nc.vector.reciprocal(rd_all[:, it : it + 1], dsum[:])
                    nc.sync.dma_start(
                        A_dram[it * 128 : (it + 1) * 128, :], A_bf[:]
                    )
                    nc.sync.dma_start_transpose(
                        AT[:, :, it * 128 : (it + 1) * 128],
                        A_dram[it * 128 : (it + 1) * 128, :],
                    )
                    # interleave previous head-pair's output-projection
                    # pieces so their Scalar drains don't bunch up
                    if hl == 2:
                        if it == 2:
                            out_transpose(0, 0, 2)
                        elif it == 5:
                            out_transpose(0, 2, 4)
                        elif it == 8:
                            out_matmul(0, 0, 8)
                        elif it == 11:
                            out_matmul(0, 8, NT)
                # recurrence: z_r = rda * (A_u @ z_{r-1}), res += z drains
                # (alpha ratios folded into the drain scales)
                zprev = None
                for r in range(ORDER):
                    rda = pzz.tile([128, NT], FP, tag=f"rda{r % 2}")
                    nc.vector.tensor_scalar_mul(
                        rda[:], rd_all[:],
                        kbc[:, r * HL + hl : r * HL + hl + 1],
                    )
                    ztmp = pzz.tile([64, N], F16, tag="ztmp")
                    for nck in range(4):
                        psz = psum.tile(
                            [64, 512], FP, tag="z", bufs=2, name="psz"
                        )
                        for jt in range(NT):
                            lhsT = (
                                v_sb[:, jt, hl * 64 : (hl + 1) * 64]
                                if r == 0
                                else zprev[:, jt, :]
                            )
                            nc.tensor.matmul(
                                psz[:],
                                lhsT,
                                AT[:, jt, nck * 512 : (nck + 1) * 512],
                                start=(jt == 0),
                                stop=(jt == NT - 1),
                            )
                        nc.scalar.copy(
                            ztmp[:, nck * 512 : (nck + 1) * 512], psz[:]
                        )
                    # znew_r = (k_r * rd) * z^T transposed back; the k_r
                    # ratio makes znew_r = alpha_r * attn^{r+1} v when
                    # chained, so res accumulation is a plain add.
                    znew = pzz.tile(
                        [128, NT, 64], F16, tag=f"z{r % 2}", name=f"znew{r}"
                    )
                    pzt = psum.tile(
                        [128, 1024], F16, tag="zt", bufs=1, name="pzt"
                    )
                    for jt in range(NT):
                        nc.tensor.transpose(
                            pzt[:, jt * 64 : (jt + 1) * 64],
                            ztmp[:, jt * 128 : (jt + 1) * 128],
                            ident_bf[0:64, 0:64],
                        )
                    for jt in range(NT):
                        nc.scalar.activation(
                            znew[:, jt, :],
                            pzt[:, jt * 64 : (jt + 1) * 64],
                            AF.Copy,
                            scale=rda[:, jt : jt + 1],
                        )
                    rslice = res_row[:, :, hl * 64 : (hl + 1) * 64]
                    nc.gpsimd.tensor_tensor(
                        rslice, rslice, znew[:], op=ALU.add
                    )
                    # next step's lhsT must be rd*z (no alpha); the alpha
                    # factor is divided back out by the k_{r+1} ratio.
                    zprev = znew
            out_transpose(1, 0, 4)
            out_matmul(1, 0, NT)

    nc.compile()
    return nc


_CACHE: dict = {}


def _in_maps(x, Wqkv, bqkv, Wout, bout, alphas_raw):
    maps = []
    for c in range(8):
        b, hg = c // 2, c % 2
        s, e = hg * 256, (hg + 1) * 256
        wqk = np.concatenate(
            [Wqkv[:, s:e], Wqkv[:, 512 + s : 512 + e]], axis=1
        )
        maps.append(
            {
                "xt": np.ascontiguousarray(x[b].T, np.float32),
                "wqk": np.ascontiguousarray(wqk, np.float32),
                "wv": np.ascontiguousarray(
                    Wqkv[:, 1024 + s : 1024 + e], np.float32
                ),
                "bqk": np.ascontiguousarray(
                    np.concatenate([bqkv[s:e], bqkv[512 + s : 512 + e]]),
                    np.float32,
                ),
                "bv": np.ascontiguousarray(
                    bqkv[None, 1024 + s : 1024 + e], np.float32
                ),
                "wout": np.ascontiguousarray(Wout[s:e, :], np.float32),
                "bout": np.ascontiguousarray(bout[None, :], np.float32),
                "araw": np.ascontiguousarray(
                    alphas_raw[:, hg * HL : (hg + 1) * HL].reshape(1, -1),
                    np.float32,
                ),
            }
        )
    return maps


def kernel(x, Wqkv, bqkv, Wout, bout, alphas_raw, _trace=False):
    x = np.asarray(x, np.float32)
    if "nc" not in _CACHE:
        _CACHE["nc"] = _build()
    nc = _CACHE["nc"]
    maps = _in_maps(
        np.asarray(x), np.asarray(Wqkv), np.asarray(bqkv),
        np.asarray(Wout), np.asarray(bout), np.asarray(alphas_raw),
    )
    kw = {}
    if _trace:
        kw = {"trace": True}
    res = run_bass_kernel_spmd(nc, maps, core_ids=list(range(8)), **kw)
    _CACHE["last_results"] = res
    out = np.empty((4, N, DIM), np.float32)
    for b in range(4):
        out[b] = res.results[2 * b]["out"] + res.results[2 * b + 1]["out"]
    return out
